# revision 8
# baseline (speedup 1.0000x reference)
"""Trainium2 Bass kernel for NearestNeighborMatcher (retrieval_knn).

Contract: kernel(**inputs) takes FULL inputs (B=8 batches), shards one batch
element per NeuronCore (8 cores, data-parallel, no collectives), and returns
the FULL output tuple (matches0, matches1, mscores0, mscores1, sim).
"""

import sys

sys.path.insert(0, "/opt/trn_rl_repo")

import numpy as np

B, N, M, D = 8, 4096, 4096, 128
RATIO_THRESH = 0.8
DIST_THRESH = 0.7
EPS = 1e-12

_CACHE = {}


def _build(n=N, m=M, d=D):
    """Build + compile the per-core Bass program.

    Per core: d0 [n,d], d1 [m,d] ->
      sim [n,m] f32, matches0 [1,n] i32, matches1 [1,m] i32,
      mscores0 [1,n] f32, mscores1 [1,m] f32.
    """
    import concourse.bass as bass
    import concourse.mybir as mybir
    from concourse import bacc, tile
    from concourse.bass import ts
    from concourse.masks import make_identity

    f32 = mybir.dt.float32
    bf16 = mybir.dt.bfloat16
    i32 = mybir.dt.int32
    u32 = mybir.dt.uint32
    u16 = mybir.dt.uint16
    Alu = mybir.AluOpType
    Act = mybir.ActivationFunctionType

    assert d == 128 and n % 128 == 0 and m % 128 == 0
    RTN = n // 128  # row tiles (dir0: rows of sim)
    RTM = m // 128  # row tiles of simT (dir1: cols of sim)

    nc = bacc.Bacc(
        "TRN2", target_bir_lowering=False, debug=False, enable_asserts=True
    )
    d0 = nc.dram_tensor("descriptors0", [n, d], f32, kind="ExternalInput")
    d1 = nc.dram_tensor("descriptors1", [m, d], f32, kind="ExternalInput")
    sim_o = nc.dram_tensor("sim", [n, m], f32, kind="ExternalOutput")
    # pre-mutual-check matches (-1 or best index), as f32; host does the
    # O(n) mutual-check index fixup + mscores
    s0 = nc.dram_tensor("prematch0", [1, n], f32, kind="ExternalOutput")
    s1 = nc.dram_tensor("prematch1", [1, m], f32, kind="ExternalOutput")

    with tile.TileContext(nc) as tc:
        import contextlib

        ctx = contextlib.ExitStack()
        with ctx:
            persist = ctx.enter_context(tc.tile_pool(name="persist", bufs=1))
            ldpool = ctx.enter_context(tc.tile_pool(name="ld", bufs=RTN + RTM))
            sqpool = ctx.enter_context(tc.tile_pool(name="sq", bufs=2))
            psum = ctx.enter_context(
                tc.tile_pool(name="psum", bufs=2, space="PSUM")
            )
            simpool = ctx.enter_context(tc.tile_pool(name="simbuf", bufs=3))
            smalls = ctx.enter_context(tc.tile_pool(name="smalls", bufs=2))

            idt = persist.tile([128, 128], bf16, tag="idt")
            make_identity(nc, idt[:])

            d0T = persist.tile([128, n], bf16, tag="d0T")
            d1T = persist.tile([128, m], bf16, tag="d1T")

            # ---------------- prologue: normalize + transpose ----------------
            def prologue(src, xT, rt_count, tag):
                lds = []
                ss = persist.tile([128, rt_count], f32, tag=f"ss_{tag}")
                for t in range(rt_count):
                    ld = ldpool.tile([128, 128], f32, tag="ld")
                    nc.sync.dma_start(ld[:], src.ap()[ts(t, 128), :])
                    lds.append(ld)
                    sq = sqpool.tile([128, 128], f32, tag="sq")
                    nc.scalar.activation(
                        sq[:], ld[:], Act.Square, accum_out=ss[:, t : t + 1]
                    )
                # batched: z = 1/max(sqrt(ss), eps), one Newton step on rsqrt
                sroot = persist.tile([128, rt_count], f32, tag=f"sr_{tag}")
                nc.scalar.activation(sroot[:], ss[:], Act.Sqrt)
                nc.vector.tensor_scalar_max(sroot[:], sroot[:], float(EPS))
                z = persist.tile([128, rt_count], f32, tag=f"z_{tag}")
                nc.vector.reciprocal(z[:], sroot[:])
                # Newton: z <- z * (1.5 - 0.5 * ss * z^2)
                t1 = smalls.tile([128, rt_count], f32, tag="nt1")
                nc.vector.tensor_mul(t1[:], z[:], z[:])
                nc.vector.tensor_mul(t1[:], t1[:], ss[:])
                nc.vector.tensor_scalar(
                    t1[:], t1[:], -0.5, 1.5, op0=Alu.mult, op1=Alu.add
                )
                nc.vector.tensor_mul(z[:], z[:], t1[:])
                for t in range(rt_count):
                    nb = sqpool.tile([128, 128], bf16, tag="nb")
                    nc.vector.tensor_scalar(
                        nb[:], lds[t][:], z[:, t : t + 1], None, op0=Alu.mult
                    )
                    pt = psum.tile([128, 128], bf16, tag="ps")
                    nc.tensor.transpose(pt[:], nb[:], idt[:])
                    nc.scalar.copy(xT[:, ts(t, 128)], pt[:])

            prologue(d0, d0T, RTN, "0")
            prologue(d1, d1T, RTM, "1")

            # ---------------- main loops: matmul + evac + top8 ----------------
            val8_0 = persist.tile([128, 8 * RTN], f32, tag="v80")
            idx8_0 = persist.tile([128, 8 * RTN], u32, tag="i80")
            val8_1 = persist.tile([128, 8 * RTM], f32, tag="v81")
            idx8_1 = persist.tile([128, 8 * RTM], u32, tag="i81")

            def direction(lhsT, rhsT, rt_count, free_n, val8, idx8, sim_dram):
                evac = min(2048, free_n)
                for rt in range(rt_count):
                    sb = simpool.tile([128, free_n], f32, tag="sb")
                    for h in range(free_n // evac):
                        ps = psum.tile([128, evac], f32, tag="ps")
                        for c in range(evac // 512):
                            nc.tensor.matmul(
                                ps[:, ts(c, 512)],
                                lhsT[:, ts(rt, 128)],
                                rhsT[:, h * evac + c * 512 : h * evac + (c + 1) * 512],
                                start=True,
                                stop=True,
                            )
                        nc.scalar.copy(sb[:, ts(h, evac)], ps[:])
                    if sim_dram is not None:
                        nc.sync.dma_start(sim_dram.ap()[ts(rt, 128), :], sb[:])
                    nc.vector.max(out=val8[:, ts(rt, 8)], in_=sb[:])
                    nc.vector.max_index(
                        out=idx8[:, ts(rt, 8)],
                        in_max=val8[:, ts(rt, 8)],
                        in_values=sb[:],
                    )

            direction(d0T, d1T, RTN, m, val8_0, idx8_0, sim_o)
            direction(d1T, d0T, RTM, n, val8_1, idx8_1, None)

            # ---------------- epilogue: thresholds + mutual check -------------
            r2 = float(RATIO_THRESH**2)
            d2 = float(DIST_THRESH**2)

            def prematch(val8, idx8, rt_count, tag):
                """-> m0f [128, rt] f32 with -1 / best-index, pre-mutual."""
                v8_3d = val8[:].rearrange("p (t k) -> p t k", k=8)
                i8_3d = idx8[:].rearrange("p (t k) -> p t k", k=8)
                va = persist.tile([128, rt_count], f32, tag=f"va_{tag}")
                vb = persist.tile([128, rt_count], f32, tag=f"vb_{tag}")
                ix = persist.tile([128, rt_count], f32, tag=f"ix_{tag}")
                nc.vector.tensor_copy(
                    va[:].rearrange("p (t o) -> p t o", o=1), v8_3d[:, :, 0:1]
                )
                nc.vector.tensor_copy(
                    vb[:].rearrange("p (t o) -> p t o", o=1), v8_3d[:, :, 1:2]
                )
                nc.vector.tensor_copy(
                    ix[:].rearrange("p (t o) -> p t o", o=1), i8_3d[:, :, 0:1]
                )
                # dist = 2*(1 - v)
                dist1 = smalls.tile([128, rt_count], f32, tag=f"dA_{tag}")
                dist2 = smalls.tile([128, rt_count], f32, tag=f"dB_{tag}")
                nc.vector.tensor_scalar(
                    dist1[:], va[:], -2.0, 2.0, op0=Alu.mult, op1=Alu.add
                )
                nc.vector.tensor_scalar(
                    dist2[:], vb[:], -2.0 * r2, 2.0 * r2, op0=Alu.mult, op1=Alu.add
                )
                mA = smalls.tile([128, rt_count], f32, tag=f"mA_{tag}")
                nc.vector.tensor_tensor(mA[:], dist1[:], dist2[:], op=Alu.is_le)
                mB = smalls.tile([128, rt_count], f32, tag=f"mB_{tag}")
                nc.vector.tensor_scalar(mB[:], dist1[:], d2, None, op0=Alu.is_le)
                nc.vector.tensor_mul(mA[:], mA[:], mB[:])
                # m = mask * (idx + 1) - 1
                mf = persist.tile([128, rt_count], f32, tag=f"mf_{tag}")
                nc.vector.tensor_scalar(ix[:], ix[:], 1.0, None, op0=Alu.add)
                nc.vector.tensor_mul(mf[:], mA[:], ix[:])
                nc.vector.tensor_scalar(mf[:], mf[:], -1.0, None, op0=Alu.add)
                return mf

            m0f = prematch(val8_0, idx8_0, RTN, "0")
            m1f = prematch(val8_1, idx8_1, RTM, "1")

            nc.sync.dma_start(
                s0.ap().rearrange("a (t p) -> p a t", p=128), m0f[:]
            )
            nc.sync.dma_start(
                s1.ap().rearrange("a (t p) -> p a t", p=128), m1f[:]
            )

    nc.compile()
    return nc


def _get_nc(n=N, m=M, d=D):
    key = (n, m, d)
    if key not in _CACHE:
        _CACHE[key] = _build(n, m, d)
    return _CACHE[key]


def _mutual_check(m0, m1):
    """Reference-equivalent mutual check on int arrays [B, n] / [B, m]."""
    i0 = np.arange(m0.shape[-1])[None, :]
    i1 = np.arange(m1.shape[-1])[None, :]
    loop0 = np.take_along_axis(m1, np.where(m0 > -1, m0, 0), axis=-1)
    loop1 = np.take_along_axis(m0, np.where(m1 > -1, m1, 0), axis=-1)
    m0n = np.where((m0 > -1) & (i0 == loop0), m0, -1)
    m1n = np.where((m1 > -1) & (i1 == loop1), m1, -1)
    return m0n, m1n


def _execute(descriptors0, descriptors1, trace=False, trace_cores=None):
    from concourse.bass_utils import run_bass_kernel_spmd

    assert descriptors0.shape == (B, N, D) and descriptors1.shape == (B, M, D)
    nc = _get_nc()
    in_maps = [
        {
            "descriptors0": np.ascontiguousarray(descriptors0[b], dtype=np.float32),
            "descriptors1": np.ascontiguousarray(descriptors1[b], dtype=np.float32),
        }
        for b in range(B)
    ]
    res = run_bass_kernel_spmd(
        nc, in_maps, core_ids=list(range(B)), trace=trace, trace_cores=trace_cores
    )
    sim = np.stack([res.results[b]["sim"] for b in range(B)])
    p0 = np.stack([res.results[b]["prematch0"].reshape(N) for b in range(B)])
    p1 = np.stack([res.results[b]["prematch1"].reshape(M) for b in range(B)])
    m0, m1 = _mutual_check(
        np.rint(p0).astype(np.int32), np.rint(p1).astype(np.int32)
    )
    ms0 = (m0 > -1).astype(np.float32)
    ms1 = (m1 > -1).astype(np.float32)
    out = (
        m0.astype(np.int32),
        m1.astype(np.int32),
        ms0,
        ms1,
        sim.astype(np.float32),
    )
    return out, res


def kernel(descriptors0: np.ndarray, descriptors1: np.ndarray):
    out, _ = _execute(descriptors0, descriptors1)
    return out


# revision 10
# speedup vs baseline: 1.7981x; 1.7981x over previous
"""Trainium2 Bass kernel for NearestNeighborMatcher (retrieval_knn).

Contract: kernel(**inputs) takes FULL inputs (B=8 batches), shards one batch
element per NeuronCore (8 cores, data-parallel, no collectives), and returns
the FULL output tuple (matches0, matches1, mscores0, mscores1, sim).

Device computes per core: normalized bf16 descriptor transposes, the two
4096x4096 similarity matmuls (sim and simT), fp32 sim written to HBM, and
per-row top-8 candidates of every 2048-wide half via the DVE MAX8 unit for
both directions. Host finishes with O(n) work: top-2 merge, ratio/distance
thresholds, argmax lookup for the (rare) mask-passing rows from the sim
output itself, and the mutual check.
"""

import sys

sys.path.insert(0, "/opt/trn_rl_repo")

import numpy as np

B, N, M, D = 8, 4096, 4096, 128
RATIO_THRESH = 0.8
DIST_THRESH = 0.7
EPS = 1e-12
HALF = 2048  # PSUM accumulation / evacuation granularity

_CACHE = {}


def _build(n=N, m=M, d=D):
    import concourse.bass as bass
    import concourse.mybir as mybir
    from concourse import bacc, tile
    from concourse.bass import ts
    from concourse.masks import make_identity

    f32 = mybir.dt.float32
    bf16 = mybir.dt.bfloat16
    Alu = mybir.AluOpType
    Act = mybir.ActivationFunctionType

    assert d == 128 and n % 128 == 0 and m % 128 == 0
    RTN, RTM = n // 128, m // 128
    half = min(HALF, m)
    assert n % half == 0 and m % half == 0
    NCH_N, NCH_M = m // half, n // half  # halves per row, per direction

    nc = bacc.Bacc(
        "TRN2", target_bir_lowering=False, debug=False, enable_asserts=True
    )
    d0 = nc.dram_tensor("descriptors0", [n, d], f32, kind="ExternalInput")
    d1 = nc.dram_tensor("descriptors1", [m, d], f32, kind="ExternalInput")
    sim_o = nc.dram_tensor("sim", [n, m], f32, kind="ExternalOutput")
    th0 = nc.dram_tensor("top8h0", [1, n * NCH_N * 8], f32, kind="ExternalOutput")
    th1 = nc.dram_tensor("top8h1", [1, m * NCH_M * 8], f32, kind="ExternalOutput")

    with tile.TileContext(nc) as tc:
        import contextlib

        ctx = contextlib.ExitStack()
        with ctx:
            persist = ctx.enter_context(tc.tile_pool(name="persist", bufs=1))
            ldpool = ctx.enter_context(tc.tile_pool(name="ld", bufs=RTN + RTM))
            sqpool = ctx.enter_context(tc.tile_pool(name="sq", bufs=2))
            psum = ctx.enter_context(
                tc.tile_pool(name="psum", bufs=2, space="PSUM")
            )
            simpool = ctx.enter_context(tc.tile_pool(name="simbuf", bufs=4))

            idt = persist.tile([128, 128], bf16, tag="idt")
            make_identity(nc, idt[:])

            d0T = persist.tile([128, n], bf16, tag="d0T")
            d1T = persist.tile([128, m], bf16, tag="d1T")

            # ---------------- prologue: normalize + transpose ----------------
            def prologue(src, xT, rt_count, tag):
                lds = []
                ss = persist.tile([128, rt_count], f32, tag=f"ss_{tag}")
                for t in range(rt_count):
                    ld = ldpool.tile([128, 128], f32, tag="ld")
                    nc.sync.dma_start(ld[:], src.ap()[ts(t, 128), :])
                    lds.append(ld)
                    sq = sqpool.tile([128, 128], f32, tag="sq")
                    nc.scalar.activation(
                        sq[:], ld[:], Act.Square, accum_out=ss[:, t : t + 1]
                    )
                # batched: z = 1/max(sqrt(ss), eps), one Newton step on rsqrt
                sroot = persist.tile([128, rt_count], f32, tag=f"sr_{tag}")
                nc.scalar.activation(sroot[:], ss[:], Act.Sqrt)
                nc.vector.tensor_scalar_max(sroot[:], sroot[:], float(EPS))
                z = persist.tile([128, rt_count], f32, tag=f"z_{tag}")
                nc.vector.reciprocal(z[:], sroot[:])
                t1 = sqpool.tile([128, rt_count], f32, tag="nt1")
                nc.vector.tensor_mul(t1[:], z[:], z[:])
                nc.vector.tensor_mul(t1[:], t1[:], ss[:])
                nc.vector.tensor_scalar(
                    t1[:], t1[:], -0.5, 1.5, op0=Alu.mult, op1=Alu.add
                )
                nc.vector.tensor_mul(z[:], z[:], t1[:])
                for t in range(rt_count):
                    nb = sqpool.tile([128, 128], bf16, tag="nb")
                    nc.scalar.activation(
                        nb[:], lds[t][:], Act.Copy, scale=z[:, t : t + 1]
                    )
                    pt = psum.tile([128, 128], bf16, tag="ps")
                    nc.tensor.transpose(pt[:], nb[:], idt[:])
                    nc.vector.tensor_copy(xT[:, ts(t, 128)], pt[:])

            prologue(d0, d0T, RTN, "0")
            prologue(d1, d1T, RTM, "1")

            h8_0 = persist.tile([128, RTN * NCH_N * 8], f32, tag="h80")
            h8_1 = persist.tile([128, RTM * NCH_M * 8], f32, tag="h81")

            # ------------- main: matmul halves + evac + max8 ------------------
            def rowtile(lhsT, rhsT, rt, nch, h8, sim_dram, sqtag):
                for h in range(nch):
                    ps = psum.tile([128, half], f32, tag="ps")
                    for c in range(half // 512):
                        off = h * half + c * 512
                        nc.tensor.matmul(
                            ps[:, ts(c, 512)],
                            lhsT[:, ts(rt, 128)],
                            rhsT[:, off : off + 512],
                            start=True,
                            stop=True,
                        )
                    sq = simpool.tile([128, half], f32, tag=sqtag)
                    nc.scalar.copy(sq[:], ps[:])
                    nc.vector.max(
                        out=h8[:, (rt * nch + h) * 8 : (rt * nch + h + 1) * 8],
                        in_=sq[:],
                    )
                    if sim_dram is not None:
                        nc.sync.dma_start(
                            sim_dram.ap()[ts(rt, 128), ts(h, half)], sq[:]
                        )

            for rt in range(max(RTN, RTM)):
                if rt < RTN:
                    rowtile(d0T, d1T, rt, NCH_N, h8_0, sim_o, "sq0")
                if rt < RTM:
                    rowtile(d1T, d0T, rt, NCH_M, h8_1, None, "sq1")

            nc.sync.dma_start(
                th0.ap().rearrange("a (t p k) -> p a t k", p=128, k=NCH_N * 8),
                h8_0[:].rearrange("p (t k) -> p t k", k=NCH_N * 8),
            )
            nc.sync.dma_start(
                th1.ap().rearrange("a (t p k) -> p a t k", p=128, k=NCH_M * 8),
                h8_1[:].rearrange("p (t k) -> p t k", k=NCH_M * 8),
            )

    nc.compile()
    return nc


def _get_nc(n=N, m=M, d=D):
    key = (n, m, d)
    if key not in _CACHE:
        _CACHE[key] = _build(n, m, d)
    return _CACHE[key]


def _find_nn_host(cand, sim_rows):
    """cand: [R, K] top candidates per row; sim_rows: callable i -> sim row.

    Returns pre-mutual matches [R] int32 (-1 or argmax index)."""
    r2 = RATIO_THRESH * RATIO_THRESH
    d2 = DIST_THRESH * DIST_THRESH
    part = np.partition(cand, cand.shape[1] - 2, axis=1)
    v1 = part[:, -1]
    v2 = part[:, -2]
    dist1 = 2.0 * (1.0 - v1)
    dist2 = 2.0 * (1.0 - v2)
    mask = (dist1 <= r2 * dist2) & (dist1 <= d2)
    out = np.full(cand.shape[0], -1, dtype=np.int32)
    for i in np.nonzero(mask)[0]:
        out[i] = int(np.argmax(sim_rows(int(i))))
    return out


def _mutual_check(m0, m1):
    i0 = np.arange(m0.shape[-1])[None, :]
    i1 = np.arange(m1.shape[-1])[None, :]
    loop0 = np.take_along_axis(m1, np.where(m0 > -1, m0, 0), axis=-1)
    loop1 = np.take_along_axis(m0, np.where(m1 > -1, m1, 0), axis=-1)
    m0n = np.where((m0 > -1) & (i0 == loop0), m0, -1)
    m1n = np.where((m1 > -1) & (i1 == loop1), m1, -1)
    return m0n, m1n


def _postprocess(res, n=N, m=M):
    half = min(HALF, m)
    nch_n, nch_m = m // half, n // half
    sim = np.stack([res.results[b]["sim"] for b in range(B)])
    m0 = np.empty((B, n), dtype=np.int32)
    m1 = np.empty((B, m), dtype=np.int32)
    for b in range(B):
        c0 = res.results[b]["top8h0"].reshape(n // 128, 128, nch_n * 8)
        c0 = c0.reshape(n, nch_n * 8)
        c1 = res.results[b]["top8h1"].reshape(m // 128, 128, nch_m * 8)
        c1 = c1.reshape(m, nch_m * 8)
        m0[b] = _find_nn_host(c0, lambda i: sim[b, i, :])
        m1[b] = _find_nn_host(c1, lambda j: sim[b, :, j])
    m0, m1 = _mutual_check(m0, m1)
    ms0 = (m0 > -1).astype(np.float32)
    ms1 = (m1 > -1).astype(np.float32)
    return (
        m0.astype(np.int32),
        m1.astype(np.int32),
        ms0,
        ms1,
        sim.astype(np.float32),
    )


def _execute(descriptors0, descriptors1, trace=False, trace_cores=None):
    from concourse.bass_utils import run_bass_kernel_spmd

    assert descriptors0.shape == (B, N, D) and descriptors1.shape == (B, M, D)
    nc = _get_nc()
    in_maps = [
        {
            "descriptors0": np.ascontiguousarray(descriptors0[b], dtype=np.float32),
            "descriptors1": np.ascontiguousarray(descriptors1[b], dtype=np.float32),
        }
        for b in range(B)
    ]
    res = run_bass_kernel_spmd(
        nc, in_maps, core_ids=list(range(B)), trace=trace, trace_cores=trace_cores
    )
    return _postprocess(res), res


def kernel(descriptors0: np.ndarray, descriptors1: np.ndarray):
    out, _ = _execute(descriptors0, descriptors1)
    return out
